# revision 1
# baseline (speedup 1.0000x reference)
"""GAT layer (single head) on Trainium2, 8 NeuronCores.

Strategy (destination-sharded):
  Launch A: per-core dense matmul producing table rows
            [h (48) | a_src | a_dst | zeros] = x @ [W | W@att_src | W@att_dst | 0].
  Host:     sort nodes by (degree, bank0-count) into 784 tiles of 128; pad each
            tile's incident edges into per-(tile,bank) slot rectangles; emit
            biased int16 source-row indices (dma_gather addresses a 65536-row
            window around a biased base pointer, so 100k rows need 2 banks).
  Launch B: per-core: dma_gather edge rows (striped over 4 SWDGE queues),
            edge softmax without max-subtraction (logits bounded), weighted
            aggregation via strided multiply+reduce, ELU, final linear,
            log_softmax.
"""
import numpy as np

N_NODES = 100_000
N_EDGES = 1_600_000
IN_CH = 128
HIDDEN = 48
OUT_CH = 16
NEG_SLOPE = 0.2

P = 128
ROW = 64                      # table row width (f32) -> 256B, dma_gather granule
CORES = 8
NODES_PER_CORE = 12544        # 98 tiles x 128
NT = 98                       # tiles per core
N_TILES = NT * CORES          # 784
N_RANKS = N_TILES * P         # 100352
TABLE_ROWS = 163840           # declared table rows (bank1 window slack)
BANK0_BASE = 32768            # biased base row for bank0 (rows [0, 65536))
BANK1_BASE = 98304            # biased base row for bank1 (rows [65536, 131072))
PAD0_TROW = 65535             # pad row inside bank0 window (biased +32767)
PAD1_TROW = 100001            # pad row inside bank1 window (biased +1697)
PAD_ASRC = -100.0             # a_src of pad rows -> exp(leaky(-100+a)) ~ 0
BATCH = 4                     # tiles per gather call pair
NQ = 4                        # SWDGE queues

_f32 = np.float32


def _build_phase_a():
    import concourse.bacc as bacc
    import concourse.mybir as mybir
    import concourse.tile as tile
    from concourse.masks import make_identity

    nc = bacc.Bacc("TRN2", target_bir_lowering=False, debug=False,
                   num_devices=CORES)
    xT = nc.dram_tensor("xT", [P, NODES_PER_CORE], mybir.dt.float32,
                        kind="ExternalInput")
    W = nc.dram_tensor("W", [IN_CH, HIDDEN], mybir.dt.float32,
                       kind="ExternalInput")
    att = nc.dram_tensor("att", [HIDDEN, 2], mybir.dt.float32,
                         kind="ExternalInput")
    hx = nc.dram_tensor("hx", [P, NT, ROW], mybir.dt.float32,
                        kind="ExternalOutput")

    with tile.TileContext(nc) as tc:
        with (
            tc.tile_pool(name="const", bufs=1) as cp,
            tc.tile_pool(name="xp", bufs=3) as xp,
            tc.tile_pool(name="st", bufs=1) as st,
            tc.tile_pool(name="ps", bufs=2, space="PSUM") as ps,
            tc.tile_pool(name="ps2", bufs=2, space="PSUM") as ps2,
        ):
            ident = cp.tile([P, P], mybir.dt.float32)
            make_identity(nc, ident[:])
            w_sb = cp.tile([IN_CH, HIDDEN], mybir.dt.float32)
            nc.sync.dma_start(out=w_sb[:], in_=W[:, :])
            att_sb = cp.tile([HIDDEN, 2], mybir.dt.float32)
            nc.sync.dma_start(out=att_sb[:], in_=att[:, :])

            # W^T via PE transpose, then Wa = (W^T)^T @ att = W @ att
            wT_ps = ps.tile([HIDDEN, IN_CH], mybir.dt.float32, space="PSUM")
            nc.tensor.transpose(out=wT_ps[:], in_=w_sb[:], identity=ident[:])
            wT_sb = cp.tile([HIDDEN, IN_CH], mybir.dt.float32)
            nc.vector.tensor_copy(out=wT_sb[:], in_=wT_ps[:])
            wa_ps = ps2.tile([P, 2], mybir.dt.float32, space="PSUM")
            nc.tensor.matmul(out=wa_ps[:], lhsT=wT_sb[:], rhs=att_sb[:],
                             start=True, stop=True)

            rhs_all = cp.tile([IN_CH, ROW], mybir.dt.float32)
            nc.vector.memset(rhs_all[:], 0.0)
            nc.vector.tensor_copy(out=rhs_all[:, 0:HIDDEN], in_=w_sb[:])
            nc.vector.tensor_copy(out=rhs_all[:, HIDDEN:HIDDEN + 2],
                                  in_=wa_ps[:])

            stage = st.tile([P, NT, ROW], mybir.dt.float32)
            for t in range(NT):
                xt = xp.tile([P, P], mybir.dt.float32, tag="xt")
                nc.sync.dma_start(out=xt[:], in_=xT[:, t * P:(t + 1) * P])
                h_ps = ps.tile([P, ROW], mybir.dt.float32, space="PSUM",
                               tag="hps")
                nc.tensor.matmul(out=h_ps[:], lhsT=xt[:], rhs=rhs_all[:],
                                 start=True, stop=True)
                nc.vector.tensor_copy(out=stage[:, t, :], in_=h_ps[:])
            nc.sync.dma_start(out=hx[:, :, :], in_=stage[:])

    nc.finalize()
    return nc


def _layout(edge_src, edge_dst):
    """Build the padded 2-bank tile layout. Returns schedule + per-core arrays."""
    E = edge_src.shape[0]
    trow = edge_src + (edge_src >= PAD0_TROW)          # table row of source
    bank = (trow >= 65536).astype(np.int8)

    deg = np.bincount(edge_dst, minlength=N_NODES)
    n1 = np.bincount(edge_dst[bank == 1], minlength=N_NODES)
    n0 = deg - n1

    # node order: group similar (deg, n0) so per-tile bank rectangles are tight
    order = np.lexsort((-n0, -deg))                    # primary -deg, then -n0
    node_at_rank = np.full(N_RANKS, -1, np.int64)
    node_at_rank[:N_NODES] = order
    rank_of_node = np.empty(N_NODES, np.int64)
    rank_of_node[order] = np.arange(N_NODES)

    r = rank_of_node[edge_dst]
    tile_g = r >> 7                                    # global tile 0..783
    p = r & 127

    # per-global-tile per-bank max counts
    n0r = np.zeros(N_RANKS, np.int64)
    n0r[:N_NODES] = n0[order]
    n1r = np.zeros(N_RANKS, np.int64)
    n1r[:N_NODES] = n1[order]
    d0_tile = n0r.reshape(N_TILES, P).max(axis=1)
    d1_tile = n1r.reshape(N_TILES, P).max(axis=1)
    # shared schedule across cores: slot b uses tiles 8b..8b+7
    D0 = d0_tile.reshape(NT, CORES).max(axis=1)
    D1 = d1_tile.reshape(NT, CORES).max(axis=1)
    D0 = np.maximum(D0, 1)
    D1 = np.maximum(D1, 1)

    # batches of tiles -> call schedule (identical for all cores)
    batches = [list(range(k, min(k + BATCH, NT))) for k in range(0, NT, BATCH)]
    calls = []          # (bank, [slot ids], ni, [col offset of each slot])
    stream_off = 0      # in idx elements
    for bt in batches:
        for bk, Dv in ((0, D0), (1, D1)):
            offs, c = [], 0
            for b in bt:
                offs.append(c)
                c += int(Dv[b])
            c += 1  # trailing all-pad column (trim guard)
            ni = c * P
            calls.append(dict(bank=bk, slots=bt, ni=ni, col_offs=offs,
                              cols=c, stream_off=stream_off))
            stream_off += ni
    total_idx = stream_off

    # per-edge position in its core's stream
    slot = tile_g // CORES
    core = tile_g % CORES
    batch_id = slot // BATCH
    pos_in_batch = slot % BATCH
    call_id = batch_id * 2 + bank                      # calls ordered A,B per batch
    call_off = np.array([c["stream_off"] for c in calls], np.int64)
    col_off_tab = np.zeros((len(calls), BATCH), np.int64)
    for ci, c in enumerate(calls):
        for j, o in enumerate(c["col_offs"]):
            col_off_tab[ci, j] = o

    # d = within-(node,bank) counter
    key = r * 2 + bank
    sidx = np.argsort(key, kind="stable")
    ks = key[sidx]
    change = np.r_[True, ks[1:] != ks[:-1]]
    gstart = np.where(change, np.arange(E), 0)
    gstart = np.maximum.accumulate(gstart)
    d = np.empty(E, np.int64)
    d[sidx] = np.arange(E) - gstart

    pos = call_off[call_id] + (col_off_tab[call_id, pos_in_batch] + d) * P + p
    biased = np.where(bank == 0, trow - BANK0_BASE, trow - BANK1_BASE)

    # per-core idx streams, pre-filled with per-position pad values
    pad_template = np.empty(total_idx, np.int16)
    for c in calls:
        padv = PAD0_TROW - BANK0_BASE if c["bank"] == 0 else PAD1_TROW - BANK1_BASE
        pad_template[c["stream_off"]:c["stream_off"] + c["ni"]] = padv
    idx_streams = []
    for ci in range(CORES):
        arr = pad_template.copy()
        m = core == ci
        arr[pos[m]] = biased[m].astype(np.int16)
        idx_streams.append(arr)

    # wrapped-16 layout per call, concatenated; replicated across 8 groups
    wrapped = []
    for arr in idx_streams:
        blocks = []
        for c in calls:
            v = arr[c["stream_off"]:c["stream_off"] + c["ni"]]
            blocks.append(v.reshape(-1, 16).T)         # [16, ni/16]
        w16 = np.concatenate(blocks, axis=1)           # [16, total/16]
        wrapped.append(np.tile(w16, (CORES, 1)).astype(np.int16))

    return dict(calls=calls, D0=D0, D1=D1, node_at_rank=node_at_rank,
                idx_wrapped=wrapped, total_idx=total_idx)


def _build_phase_b(calls, D0, D1):
    import concourse.bacc as bacc
    import concourse.mybir as mybir
    import concourse.tile as tile
    from concourse.masks import make_identity

    AL = mybir.AluOpType
    AF = mybir.ActivationFunctionType
    total16 = sum(c["ni"] for c in calls) // 16

    nc = bacc.Bacc("TRN2", target_bir_lowering=False, debug=False,
                   num_devices=CORES, num_swdge_queues=NQ)
    table = nc.dram_tensor("table", [TABLE_ROWS, ROW], mybir.dt.float32,
                           kind="ExternalInput")
    idxs = nc.dram_tensor("idxs", [P, total16], mybir.dt.int16,
                          kind="ExternalInput")
    adst = nc.dram_tensor("adst", [P, NT], mybir.dt.float32,
                          kind="ExternalInput")
    biasr = nc.dram_tensor("biasr", [P, HIDDEN], mybir.dt.float32,
                           kind="ExternalInput")
    linW = nc.dram_tensor("linW", [HIDDEN, OUT_CH], mybir.dt.float32,
                          kind="ExternalInput")
    linbr = nc.dram_tensor("linbr", [P, OUT_CH], mybir.dt.float32,
                           kind="ExternalInput")
    outz = nc.dram_tensor("outz", [P, NT, OUT_CH], mybir.dt.float32,
                          kind="ExternalOutput")

    bank_slice = {0: (BANK0_BASE, BANK0_BASE + 65536),
                  1: (BANK1_BASE, TABLE_ROWS)}

    with tile.TileContext(nc) as tc:
        with (
            tc.tile_pool(name="const", bufs=1) as cp,
            tc.tile_pool(name="ga", bufs=2) as gap,
            tc.tile_pool(name="gb", bufs=2) as gbp,
            tc.tile_pool(name="sc", bufs=3) as sp,
            tc.tile_pool(name="ps", bufs=2, space="PSUM") as ps,
            tc.tile_pool(name="ps2", bufs=2, space="PSUM") as ps2,
        ):
            ident = cp.tile([P, P], mybir.dt.float32)
            make_identity(nc, ident[:])
            idx_sb = cp.tile([P, total16], mybir.dt.int16)
            nc.sync.dma_start(out=idx_sb[:], in_=idxs[:, :])
            adst_sb = cp.tile([P, NT], mybir.dt.float32)
            nc.sync.dma_start(out=adst_sb[:], in_=adst[:, :])
            bias_sb = cp.tile([P, HIDDEN], mybir.dt.float32)
            nc.sync.dma_start(out=bias_sb[:], in_=biasr[:, :])
            linW_sb = cp.tile([HIDDEN, OUT_CH], mybir.dt.float32)
            nc.sync.dma_start(out=linW_sb[:], in_=linW[:, :])
            linb_sb = cp.tile([P, OUT_CH], mybir.dt.float32)
            nc.sync.dma_start(out=linb_sb[:], in_=linbr[:, :])
            ostage = cp.tile([P, NT, OUT_CH], mybir.dt.float32)

            qn = 0
            for k in range(0, len(calls), 2):
                cA, cB = calls[k], calls[k + 1]
                g = {}
                for c in (cA, cB):
                    pool = gap if c["bank"] == 0 else gbp
                    gt = pool.tile([P, c["cols"], ROW], mybir.dt.float32,
                                   tag=f"g{c['bank']}")
                    off16 = c["stream_off"] // 16
                    lo, hi = bank_slice[c["bank"]]
                    nc.gpsimd.dma_gather(
                        gt[:], table[lo:hi, :],
                        idx_sb[:, off16:off16 + c["ni"] // 16],
                        c["ni"], c["ni"], ROW,
                        single_packet=False, queue_num=qn % NQ)
                    qn += 1
                    g[c["bank"]] = gt

                for j, b in enumerate(cA["slots"]):
                    d0, d1 = int(D0[b]), int(D1[b])
                    a0 = cA["col_offs"][j]
                    a1 = cB["col_offs"][j]
                    gA, gB = g[0], g[1]
                    dt_ = d0 + d1

                    # leaky_relu(a_src + a_dst) = max(t, 0.2*t), t = a_src + a_dst
                    logit = sp.tile([P, dt_], mybir.dt.float32, tag="logit")
                    lu = sp.tile([P, dt_], mybir.dt.float32, tag="lu")
                    for gsb, aoff, dd, loff in ((gA, a0, d0, 0),
                                                (gB, a1, d1, d0)):
                        nc.vector.tensor_scalar_add(
                            out=logit[:, loff:loff + dd],
                            in0=gsb[:, aoff:aoff + dd, HIDDEN],
                            scalar1=adst_sb[:, b:b + 1])
                        nc.vector.tensor_scalar(
                            out=lu[:, loff:loff + dd],
                            in0=gsb[:, aoff:aoff + dd, HIDDEN],
                            scalar1=adst_sb[:, b:b + 1],
                            scalar2=NEG_SLOPE, op0=AL.add, op1=AL.mult)
                    nc.vector.tensor_tensor(out=logit[:], in0=logit[:],
                                            in1=lu[:], op=AL.max)
                    w = sp.tile([P, dt_], mybir.dt.float32, tag="w")
                    denom = sp.tile([P, 1], mybir.dt.float32, tag="den")
                    nc.scalar.activation(out=w[:], in_=logit[:], func=AF.Exp,
                                         accum_out=denom[:])

                    nc.vector.tensor_tensor(
                        out=gA[:, a0:a0 + d0, 0:HIDDEN],
                        in0=gA[:, a0:a0 + d0, 0:HIDDEN],
                        in1=w[:, 0:d0][:, :, None].broadcast_to([P, d0, HIDDEN]),
                        op=AL.mult)
                    nc.vector.tensor_tensor(
                        out=gB[:, a1:a1 + d1, 0:HIDDEN],
                        in0=gB[:, a1:a1 + d1, 0:HIDDEN],
                        in1=w[:, d0:dt_][:, :, None].broadcast_to([P, d1, HIDDEN]),
                        op=AL.mult)

                    agg = sp.tile([P, HIDDEN], mybir.dt.float32, tag="agg")
                    agg2 = sp.tile([P, HIDDEN], mybir.dt.float32, tag="agg2")
                    nc.vector.tensor_reduce(
                        out=agg[:],
                        in_=gA[:, a0:a0 + d0, 0:HIDDEN].rearrange("p d c -> p c d"),
                        axis=mybir.AxisListType.X, op=AL.add)
                    nc.vector.tensor_reduce(
                        out=agg2[:],
                        in_=gB[:, a1:a1 + d1, 0:HIDDEN].rearrange("p d c -> p c d"),
                        axis=mybir.AxisListType.X, op=AL.add)
                    nc.vector.tensor_tensor(out=agg[:], in0=agg[:], in1=agg2[:],
                                            op=AL.add)

                    rden = sp.tile([P, 1], mybir.dt.float32, tag="rden")
                    nc.vector.reciprocal(rden[:], denom[:])
                    nc.vector.tensor_scalar_mul(out=agg[:], in0=agg[:],
                                                scalar1=rden[:])
                    nc.vector.tensor_tensor(out=agg[:], in0=agg[:],
                                            in1=bias_sb[:], op=AL.add)
                    # ELU: elu(y) = max(y,0) + exp(min(y,0)) - 1
                    tmin = sp.tile([P, HIDDEN], mybir.dt.float32, tag="tmin")
                    nc.vector.tensor_scalar_min(out=tmin[:], in0=agg[:],
                                                scalar1=0.0)
                    nc.scalar.activation(out=tmin[:], in_=tmin[:], func=AF.Exp)
                    nc.vector.tensor_scalar_max(out=agg[:], in0=agg[:],
                                                scalar1=0.0)
                    nc.vector.tensor_scalar(out=tmin[:], in0=tmin[:],
                                            scalar1=1.0, scalar2=None,
                                            op0=AL.subtract)
                    nc.vector.tensor_tensor(out=agg[:], in0=agg[:], in1=tmin[:],
                                            op=AL.add)

                    # z = elu_out @ linW + linb, then log_softmax
                    yT_ps = ps.tile([HIDDEN, P], mybir.dt.float32, space="PSUM",
                                    tag="yT")
                    nc.tensor.transpose(out=yT_ps[:], in_=agg[:],
                                        identity=ident[:])
                    yT_sb = sp.tile([HIDDEN, P], mybir.dt.float32, tag="yT_sb")
                    nc.vector.tensor_copy(out=yT_sb[:], in_=yT_ps[:])
                    z_ps = ps2.tile([P, OUT_CH], mybir.dt.float32, space="PSUM",
                                    tag="z")
                    nc.tensor.matmul(out=z_ps[:], lhsT=yT_sb[:], rhs=linW_sb[:],
                                     start=True, stop=True)
                    zy = sp.tile([P, OUT_CH], mybir.dt.float32, tag="zy")
                    nc.vector.tensor_tensor(out=zy[:], in0=z_ps[:],
                                            in1=linb_sb[:], op=AL.add)
                    nm = sp.tile([P, 1], mybir.dt.float32, tag="nm")
                    nc.vector.tensor_reduce(out=nm[:], in_=zy[:],
                                            axis=mybir.AxisListType.X,
                                            op=AL.max)
                    nc.vector.tensor_scalar_mul(out=nm[:], in0=nm[:],
                                                scalar1=-1.0)
                    es = sp.tile([P, OUT_CH], mybir.dt.float32, tag="es")
                    s = sp.tile([P, 1], mybir.dt.float32, tag="s")
                    nc.scalar.activation(out=es[:], in_=zy[:], func=AF.Exp,
                                         bias=nm[:], accum_out=s[:])
                    ls = sp.tile([P, 1], mybir.dt.float32, tag="ls")
                    nc.scalar.activation(out=ls[:], in_=s[:], func=AF.Ln)
                    nc.vector.tensor_tensor(out=nm[:], in0=nm[:], in1=ls[:],
                                            op=AL.subtract)
                    nc.vector.tensor_scalar_add(out=ostage[:, b, :], in0=zy[:],
                                                scalar1=nm[:])
            nc.sync.dma_start(out=outz[:, :, :], in_=ostage[:])

    nc.finalize()
    return nc


EXEC_TIMES = []


def kernel(x, edge_index, W, att_src, att_dst, gat_bias, lin_W, lin_b):
    import os
    from concourse.bass_utils import run_bass_kernel_spmd
    trace = os.environ.get("GAT_TRACE") == "1"

    x = np.asarray(x, _f32)
    edge_index = np.asarray(edge_index)
    W = np.asarray(W, _f32)
    att_src = np.asarray(att_src, _f32)
    att_dst = np.asarray(att_dst, _f32)
    gat_bias = np.asarray(gat_bias, _f32)
    lin_W = np.asarray(lin_W, _f32)
    lin_b = np.asarray(lin_b, _f32)

    # ---- launch A: table rows -------------------------------------------
    nc_a = _build_phase_a()
    xT = np.ascontiguousarray(x.T)                    # [128, 100000]
    att2 = np.stack([att_src, att_dst], axis=1)       # [48, 2]
    in_maps_a = []
    for c in range(CORES):
        sh = np.zeros((P, NODES_PER_CORE), _f32)
        sh[:, :12500] = xT[:, c * 12500:(c + 1) * 12500]
        in_maps_a.append({"xT": sh, "W": W, "att": att2})
    res_a = run_bass_kernel_spmd(nc_a, in_maps_a, core_ids=list(range(CORES)), trace=trace)
    EXEC_TIMES.append(("phase_a", res_a.exec_time_ns))
    hx = np.zeros((N_NODES, ROW), _f32)
    for c in range(CORES):
        o = res_a.results[c]["hx"]                    # [128, 98, 64]
        hx[c * 12500:(c + 1) * 12500] = (
            o.transpose(1, 0, 2).reshape(NODES_PER_CORE, ROW)[:12500])

    # ---- host: edge layout ----------------------------------------------
    src = np.concatenate([edge_index[0], np.arange(N_NODES, dtype=np.int64)])
    dst = np.concatenate([edge_index[1], np.arange(N_NODES, dtype=np.int64)])
    lay = _layout(src.astype(np.int64), dst.astype(np.int64))

    table = np.zeros((TABLE_ROWS, ROW), _f32)
    table[0:PAD0_TROW] = hx[0:PAD0_TROW]
    table[PAD0_TROW, HIDDEN] = PAD_ASRC
    table[PAD0_TROW + 1:N_NODES + 1] = hx[PAD0_TROW:]
    table[PAD1_TROW, HIDDEN] = PAD_ASRC

    a_dst_vec = hx[:, HIDDEN + 1]
    node_at_rank = lay["node_at_rank"]
    adst_cores = []
    for c in range(CORES):
        arr = np.zeros((P, NT), _f32)
        for b in range(NT):
            tg = b * CORES + c
            nodes = node_at_rank[tg * P:(tg + 1) * P]
            valid = nodes >= 0
            arr[valid, b] = a_dst_vec[nodes[valid]]
        adst_cores.append(arr)

    # ---- launch B --------------------------------------------------------
    nc_b = _build_phase_b(lay["calls"], lay["D0"], lay["D1"])
    biasr = np.tile(gat_bias[None, :], (P, 1)).astype(_f32)
    linbr = np.tile(lin_b[None, :], (P, 1)).astype(_f32)
    in_maps_b = []
    for c in range(CORES):
        in_maps_b.append({
            "table": table, "idxs": lay["idx_wrapped"][c],
            "adst": adst_cores[c], "biasr": biasr,
            "linW": lin_W, "linbr": linbr,
        })
    res_b = run_bass_kernel_spmd(nc_b, in_maps_b, core_ids=list(range(CORES)), trace=trace)
    EXEC_TIMES.append(("phase_b", res_b.exec_time_ns))

    out = np.zeros((N_NODES, OUT_CH), _f32)
    for c in range(CORES):
        oz = res_b.results[c]["outz"]                 # [128, 98, 16]
        for b in range(NT):
            tg = b * CORES + c
            nodes = node_at_rank[tg * P:(tg + 1) * P]
            valid = nodes >= 0
            out[nodes[valid]] = oz[valid, b, :]
    return out



# revision 2
# speedup vs baseline: 1.0022x; 1.0022x over previous
"""GAT layer (single head) on Trainium2, 8 NeuronCores — v2.

exp(leaky_relu(t)) = max(exp(t), exp(0.2 t)); each side separates into
per-src x per-dst factors. Host splits edges by sign(t) using phase-A
a-values, so each edge weight is (table-premultiplied per-src factor) x
(per-dst factor applied after aggregation).

Phase A: h = x@W; emits bf16 tables rowP=[exp(a_src)*h|exp(a_src)],
         rowM=[exp(.2 a_src)*h|exp(.2 a_src)] and per-node a/q values.
Host:    edges -> (dst-tile, bank, sign) sections, 128-edge blocks,
         shared max-over-cores schedule; int16 idx + bf16 dst-local
         streams (pad slots: dstl=-1 kills their contribution).
Phase B: dma_gather rows; per block one DVE is_equal one-hot + one PE
         matmul accumulating [dst,49] in PSUM (col 48 = denominator);
         per-tile q-scaling + affine self-loop term; batched ELU,
         linear, log_softmax.
"""
import numpy as np
import ml_dtypes

N_NODES = 100_000
N_EDGES = 1_600_000
IN_CH = 128
HIDDEN = 48
OUT_CH = 16
NEG_SLOPE = 0.2

P = 128
CORES = 8
NA = 12500                    # phase-A nodes per core
NT_A = 98                     # phase-A tiles per core
GT = 782                      # global dst tiles (781*128 + 32)
NT = 98                       # phase-B tile slots per core
ROWE = 128                    # table row elems (bf16) -> 256B
RU = 49                       # used row elems: 48 ch + denom
BANK_BASE = (32768, 98304)
KT = 8                        # tiles per gather batch
NQ = 4

_f32 = np.float32
_bf16 = ml_dtypes.bfloat16


# ---------------------------------------------------------------- phase A
def _build_phase_a():
    import concourse.bacc as bacc
    import concourse.mybir as mybir
    import concourse.tile as tile
    from concourse.masks import make_identity

    AL = mybir.AluOpType
    AF = mybir.ActivationFunctionType

    nc = bacc.Bacc("TRN2", target_bir_lowering=False, debug=False,
                   num_devices=CORES)
    xT = nc.dram_tensor("xT", [P, NT_A * P], mybir.dt.float32,
                        kind="ExternalInput")
    W = nc.dram_tensor("W", [IN_CH, HIDDEN], mybir.dt.float32,
                       kind="ExternalInput")
    att = nc.dram_tensor("att", [HIDDEN, 2], mybir.dt.float32,
                         kind="ExternalInput")
    rowP = nc.dram_tensor("rowP", [P, NT_A, RU], mybir.dt.bfloat16,
                          kind="ExternalOutput")
    rowM = nc.dram_tensor("rowM", [P, NT_A, RU], mybir.dt.bfloat16,
                          kind="ExternalOutput")
    avals = nc.dram_tensor("avals", [P, NT_A, 2], mybir.dt.float32,
                           kind="ExternalOutput")
    qvals = nc.dram_tensor("qvals", [P, NT_A, 2], mybir.dt.float32,
                           kind="ExternalOutput")

    NCHUNK = 7
    CH = NT_A // NCHUNK

    with tile.TileContext(nc) as tc:
        with (
            tc.tile_pool(name="const", bufs=1) as cp,
            tc.tile_pool(name="xp", bufs=2) as xp,
            tc.tile_pool(name="ps", bufs=2, space="PSUM") as ps,
            tc.tile_pool(name="ps2", bufs=2, space="PSUM") as ps2,
        ):
            ident = cp.tile([P, P], mybir.dt.float32)
            make_identity(nc, ident[:])
            w_sb = cp.tile([IN_CH, HIDDEN], mybir.dt.float32)
            nc.sync.dma_start(out=w_sb[:], in_=W[:, :])
            att_sb = cp.tile([HIDDEN, 2], mybir.dt.float32)
            nc.sync.dma_start(out=att_sb[:], in_=att[:, :])

            wT_ps = ps.tile([HIDDEN, IN_CH], mybir.dt.float32, space="PSUM")
            nc.tensor.transpose(out=wT_ps[:], in_=w_sb[:], identity=ident[:])
            wT_sb = cp.tile([HIDDEN, IN_CH], mybir.dt.float32)
            nc.vector.tensor_copy(out=wT_sb[:], in_=wT_ps[:])
            wa_ps = ps2.tile([P, 2], mybir.dt.float32, space="PSUM")
            nc.tensor.matmul(out=wa_ps[:], lhsT=wT_sb[:], rhs=att_sb[:],
                             start=True, stop=True)
            rhs50 = cp.tile([IN_CH, HIDDEN + 2], mybir.dt.float32)
            nc.vector.tensor_copy(out=rhs50[:, 0:HIDDEN], in_=w_sb[:])
            nc.vector.tensor_copy(out=rhs50[:, HIDDEN:HIDDEN + 2],
                                  in_=wa_ps[:])

            hstage = cp.tile([P, NT_A, HIDDEN + 2], mybir.dt.float32)
            asrc_t = hstage[:, :, HIDDEN]
            adst_t = hstage[:, :, HIDDEN + 1]

            GRP = 7   # tiles per PSUM tile: 7*50=350 <= 512 f32 bank
            for ck in range(NCHUNK):
                xt = xp.tile([P, CH * P], mybir.dt.float32, tag="xt")
                nc.sync.dma_start(out=xt[:],
                                  in_=xT[:, ck * CH * P:(ck + 1) * CH * P])
                for g0 in range(0, CH, GRP):
                    gn = min(GRP, CH - g0)
                    h_ps = ps.tile([P, GRP, HIDDEN + 2], mybir.dt.float32,
                                   space="PSUM", tag="hps")
                    for j in range(gn):
                        nc.tensor.matmul(out=h_ps[:, j, :],
                                         lhsT=xt[:, (g0 + j) * P:(g0 + j + 1) * P],
                                         rhs=rhs50[:], start=True, stop=True)
                    t = ck * CH + g0
                    nc.vector.tensor_copy(out=hstage[:, t:t + gn, :],
                                          in_=h_ps[:, 0:gn, :])

            Pt = cp.tile([P, NT_A], mybir.dt.float32)
            P2t = cp.tile([P, NT_A], mybir.dt.float32)
            qt = cp.tile([P, NT_A], mybir.dt.float32)
            q2t = cp.tile([P, NT_A], mybir.dt.float32)
            tmp = cp.tile([P, NT_A], mybir.dt.float32)
            nc.scalar.activation(out=Pt[:], in_=asrc_t, func=AF.Exp)
            nc.vector.tensor_scalar(out=tmp[:], in0=asrc_t,
                                    scalar1=NEG_SLOPE, scalar2=None,
                                    op0=AL.mult)
            nc.scalar.activation(out=P2t[:], in_=tmp[:], func=AF.Exp)
            nc.scalar.activation(out=qt[:], in_=adst_t, func=AF.Exp)
            nc.vector.tensor_scalar(out=tmp[:], in0=adst_t,
                                    scalar1=NEG_SLOPE, scalar2=None,
                                    op0=AL.mult)
            nc.scalar.activation(out=q2t[:], in_=tmp[:], func=AF.Exp)

            rP = cp.tile([P, NT_A, RU], mybir.dt.bfloat16)
            rM = cp.tile([P, NT_A, RU], mybir.dt.bfloat16)
            nc.vector.tensor_tensor(
                out=rP[:, :, 0:HIDDEN], in0=hstage[:, :, 0:HIDDEN],
                in1=Pt[:, :, None].broadcast_to([P, NT_A, HIDDEN]),
                op=AL.mult)
            nc.vector.tensor_copy(out=rP[:, :, HIDDEN], in_=Pt[:])
            nc.vector.tensor_tensor(
                out=rM[:, :, 0:HIDDEN], in0=hstage[:, :, 0:HIDDEN],
                in1=P2t[:, :, None].broadcast_to([P, NT_A, HIDDEN]),
                op=AL.mult)
            nc.vector.tensor_copy(out=rM[:, :, HIDDEN], in_=P2t[:])
            nc.sync.dma_start(out=rowP[:, :, :], in_=rP[:])
            nc.sync.dma_start(out=rowM[:, :, :], in_=rM[:])

            av = cp.tile([P, NT_A, 2], mybir.dt.float32)
            nc.vector.tensor_copy(out=av[:, :, 0], in_=asrc_t)
            nc.vector.tensor_copy(out=av[:, :, 1], in_=adst_t)
            nc.sync.dma_start(out=avals[:, :, :], in_=av[:])
            qv = cp.tile([P, NT_A, 2], mybir.dt.float32)
            nc.vector.tensor_copy(out=qv[:, :, 0], in_=qt[:])
            nc.vector.tensor_copy(out=qv[:, :, 1], in_=q2t[:])
            nc.sync.dma_start(out=qvals[:, :, :], in_=qv[:])

    nc.finalize()
    return nc


# ---------------------------------------------------------------- layout
def _layout2(src, dst, sign):
    """Shared-schedule edge layout. Returns schedule + per-core streams."""
    tg = dst >> 7                                     # global dst tile
    pl = (dst & 127).astype(np.int64)
    bank = (src >= 65536).astype(np.int64)
    sec = bank * 2 + (1 - sign.astype(np.int64))      # 0:b0+,1:b0-,2:b1+,3:b1-

    # per-(global tile, sec) counts -> blocks
    keyts = tg * 4 + sec
    cnt_t = np.bincount(keyts, minlength=GT * 4).reshape(GT, 4)
    nblk_t = (cnt_t + 127) >> 7                       # [GT, 4]

    # assign tiles to (core, slot): sort by block profile, deal rows of 8
    prof = nblk_t[:, 0] * 1000000 + nblk_t[:, 1] * 10000 \
        + nblk_t[:, 2] * 100 + nblk_t[:, 3]
    order = np.argsort(-prof, kind="stable")          # [GT]
    slot_of_tile = np.empty(GT, np.int64)
    core_of_tile = np.empty(GT, np.int64)
    for s in range((GT + CORES - 1) // CORES):
        grp = order[s * CORES:(s + 1) * CORES]
        slot_of_tile[grp] = s
        core_of_tile[grp] = np.arange(len(grp))
    NSLOT = (GT + CORES - 1) // CORES                 # 98
    assert NSLOT == NT

    # shared schedule: max blocks across the <=8 tiles of each slot
    nblk_sh = np.zeros((NT, 4), np.int64)
    for t in range(GT):
        s = slot_of_tile[t]
        nblk_sh[s] = np.maximum(nblk_sh[s], nblk_t[t])

    col_base = np.zeros((NT, 4), np.int64)
    calls = []                                        # dicts: sec, col0, cols
    col = 0
    NBATCH = (NT + KT - 1) // KT
    for bt in range(NBATCH):
        t0, t1 = bt * KT, min((bt + 1) * KT, NT)
        for s4 in range(4):
            ch = 0
            for t in range(t0, t1):
                col_base[t, s4] = col + ch
                ch += int(nblk_sh[t, s4])
            if ch:
                calls.append(dict(sec=s4, col0=col, cols=ch))
            col += ch
    total_cols = col

    # edge slot positions (within its (tile, sec) section, shared geometry)
    core = core_of_tile[tg]
    slot = slot_of_tile[tg]
    key = ((core * NT + slot) * 4 + sec)
    E = src.shape[0]
    order_e = np.lexsort((src, key))
    ks = key[order_e]
    change = np.r_[True, ks[1:] != ks[:-1]]
    gstart = np.where(change, np.arange(E), 0)
    gstart = np.maximum.accumulate(gstart)
    within = np.empty(E, np.int64)
    within[order_e] = np.arange(E) - gstart

    ecol = col_base[slot, sec] + (within >> 7)
    epos = ecol * P + (within & 127)
    biased = np.where(bank == 0, src - BANK_BASE[0], src - BANK_BASE[1])

    idx_streams, dstl_streams = [], []
    call_bounds = [(c["col0"] * P, (c["col0"] + c["cols"]) * P)
                   for c in calls]
    for c in range(CORES):
        idx = np.zeros(total_cols * P, np.int16)
        dstl = np.full((P, total_cols), -1.0, _bf16)
        m = core == c
        idx[epos[m]] = biased[m].astype(np.int16)
        dstl[(epos[m] & 127), (epos[m] >> 7)] = pl[m].astype(_bf16)
        blocks = [idx[a:b].reshape(-1, 16).T for a, b in call_bounds]
        w16 = np.concatenate(blocks, axis=1)
        idx_streams.append(np.tile(w16, (8, 1)).astype(np.int16))
        dstl_streams.append(dstl)

    node_of = np.full((CORES, NT, P), -1, np.int64)
    for t in range(GT):
        n0, n1 = t * P, min(t * P + P, N_NODES)
        node_of[core_of_tile[t], slot_of_tile[t], :n1 - n0] = \
            np.arange(n0, n1)

    return dict(calls=calls, col_base=col_base, nblk_sh=nblk_sh,
                total_cols=total_cols, idx=idx_streams, dstl=dstl_streams,
                node_of=node_of)


# ---------------------------------------------------------------- phase B
def _build_phase_b(calls, col_base, nblk_sh, total_cols):
    import concourse.bacc as bacc
    import concourse.mybir as mybir
    import concourse.tile as tile
    from concourse.masks import make_identity

    AL = mybir.AluOpType
    AF = mybir.ActivationFunctionType
    total16 = total_cols * P // 16

    nc = bacc.Bacc("TRN2", target_bir_lowering=False, debug=False,
                   num_devices=CORES, num_swdge_queues=NQ)
    tblP = nc.dram_tensor("tblP", [N_NODES, ROWE], mybir.dt.bfloat16,
                          kind="ExternalInput")
    tblM = nc.dram_tensor("tblM", [N_NODES, ROWE], mybir.dt.bfloat16,
                          kind="ExternalInput")
    idxs = nc.dram_tensor("idxs", [P, total16], mybir.dt.int16,
                          kind="ExternalInput")
    dstlt = nc.dram_tensor("dstl", [P, total_cols], mybir.dt.bfloat16,
                           kind="ExternalInput")
    qst = nc.dram_tensor("qst", [P, NT, 2], mybir.dt.float32,
                         kind="ExternalInput")
    selfR = nc.dram_tensor("selfR", [P, NT, RU], mybir.dt.bfloat16,
                           kind="ExternalInput")
    qefft = nc.dram_tensor("qeffh", [P, NT], mybir.dt.float32,
                           kind="ExternalInput")
    biasr = nc.dram_tensor("biasr", [P, HIDDEN], mybir.dt.float32,
                           kind="ExternalInput")
    linWt = nc.dram_tensor("linW", [HIDDEN, OUT_CH], mybir.dt.float32,
                           kind="ExternalInput")
    linbr = nc.dram_tensor("linbr", [P, OUT_CH], mybir.dt.float32,
                           kind="ExternalInput")
    outz = nc.dram_tensor("outz", [P, NT, OUT_CH], mybir.dt.float32,
                          kind="ExternalOutput")

    def win(s4):
        tbl = tblP if s4 in (0, 2) else tblM
        base = BANK_BASE[0] if s4 < 2 else BANK_BASE[1]
        return tbl[base:N_NODES, :]

    with tile.TileContext(nc) as tc:
        with (
            tc.tile_pool(name="const", bufs=1) as cp,
            tc.tile_pool(name="ix", bufs=2) as ixp,
            tc.tile_pool(name="g0", bufs=2) as gp0,
            tc.tile_pool(name="g1", bufs=2) as gp1,
            tc.tile_pool(name="g2", bufs=2) as gp2,
            tc.tile_pool(name="g3", bufs=2) as gp3,
            tc.tile_pool(name="m0a", bufs=2) as mp0,
            tc.tile_pool(name="m0b", bufs=2) as mp1,
            tc.tile_pool(name="m0c", bufs=2) as mp2,
            tc.tile_pool(name="m0d", bufs=2) as mp3,
            tc.tile_pool(name="sc", bufs=4) as sp,
            tc.tile_pool(name="big", bufs=1) as bigp,
            tc.tile_pool(name="pp", bufs=2, space="PSUM") as ppp,
            tc.tile_pool(name="pn", bufs=2, space="PSUM") as ppn,
            tc.tile_pool(name="py", bufs=2, space="PSUM") as pyp,
            tc.tile_pool(name="pz", bufs=2, space="PSUM") as pzp,
        ):
            ident = cp.tile([P, P], mybir.dt.float32)
            make_identity(nc, ident[:])
            ioi = sp.tile([P, P], mybir.dt.int32, tag="ioi")
            nc.gpsimd.iota(ioi[:], pattern=[[1, P]], base=0,
                           channel_multiplier=0)
            iota = cp.tile([P, P], mybir.dt.bfloat16)
            nc.vector.tensor_copy(out=iota[:], in_=ioi[:])

            dstl_sb = cp.tile([P, total_cols], mybir.dt.bfloat16)
            nc.sync.dma_start(out=dstl_sb[:], in_=dstlt[:, :])
            q_sb = cp.tile([P, NT, 2], mybir.dt.float32)
            nc.sync.dma_start(out=q_sb[:], in_=qst[:, :, :])
            bias_sb = cp.tile([P, HIDDEN], mybir.dt.float32)
            nc.sync.dma_start(out=bias_sb[:], in_=biasr[:, :])
            linW_sb = cp.tile([HIDDEN, OUT_CH], mybir.dt.float32)
            nc.sync.dma_start(out=linW_sb[:], in_=linWt[:, :])
            linb_sb = cp.tile([P, OUT_CH], mybir.dt.float32)
            nc.sync.dma_start(out=linb_sb[:], in_=linbr[:, :])

            rEff = cp.tile([P, NT, RU], mybir.dt.bfloat16)
            nc.sync.dma_start(out=rEff[:], in_=selfR[:, :, :])
            qeff = cp.tile([P, NT], mybir.dt.float32)
            nc.sync.dma_start(out=qeff[:], in_=qefft[:, :])

            aggbig = cp.tile([P, NT, RU], mybir.dt.float32)
            zst = cp.tile([P, NT, OUT_CH], mybir.dt.float32)

            gpools = {0: gp0, 1: gp1, 2: gp2, 3: gp3}
            mpools = {0: mp0, 1: mp1, 2: mp2, 3: mp3}

            def emit_sign(t, g, sgn, ntot, nbl, acc, tmp2, first):
                pool = ppp if sgn == 0 else ppn
                pst = pool.tile([P, RU], mybir.dt.float32, space="PSUM",
                                tag="pos" if sgn == 0 else "neg")
                done = 0
                for s4 in (sgn, sgn + 2):
                    for b in range(nbl[s4]):
                        gt, c0, chunks = g[s4]
                        rc = col_base[t, s4] + b - c0
                        m0b, mrow = chunks[rc]
                        nc.tensor.matmul(
                            out=pst[:], lhsT=m0b[:, mrow, :],
                            rhs=gt[:, rc, 0:RU],
                            start=(done == 0), stop=(done == ntot - 1))
                        done += 1
                qcol = 0 if sgn == 0 else 1
                if first:
                    nc.vector.tensor_scalar_mul(
                        out=acc[:], in0=pst[:],
                        scalar1=q_sb[:, t, qcol:qcol + 1])
                else:
                    nc.vector.tensor_scalar_mul(
                        out=tmp2[:], in0=pst[:],
                        scalar1=q_sb[:, t, qcol:qcol + 1])
                    nc.vector.tensor_tensor(out=acc[:], in0=acc[:],
                                            in1=tmp2[:], op=AL.add)

            def emit_tile(t, g):
                nbl = [int(nblk_sh[t, s4]) for s4 in range(4)]
                npos = nbl[0] + nbl[2]
                nneg = nbl[1] + nbl[3]
                acc = sp.tile([P, RU], mybir.dt.float32, tag="acc")
                tmp2 = sp.tile([P, RU], mybir.dt.float32, tag="tmp2")
                first = True
                for sgn in (0, 1):
                    ntot = npos if sgn == 0 else nneg
                    if ntot == 0:
                        continue
                    emit_sign(t, g, sgn, ntot, nbl, acc, tmp2, first)
                    first = False
                if first:
                    nc.vector.memset(acc[:], 0.0)
                nc.vector.tensor_scalar_mul(out=tmp2[:], in0=rEff[:, t, :],
                                            scalar1=qeff[:, t:t + 1])
                nc.vector.tensor_tensor(out=aggbig[:, t, :], in0=acc[:],
                                        in1=tmp2[:], op=AL.add)
            NBATCH = (NT + KT - 1) // KT
            ci = 0
            qn = 0
            off16 = 0
            for bt in range(NBATCH):
                t0, t1 = bt * KT, min((bt + 1) * KT, NT)
                bcols = int(sum(nblk_sh[t, s4] for t in range(t0, t1)
                                for s4 in range(4)))
                b16 = bcols * P // 16
                idx_t = ixp.tile([P, b16], mybir.dt.int16, tag="idx")
                nc.sync.dma_start(out=idx_t[:],
                                  in_=idxs[:, off16:off16 + b16])
                l16 = 0
                g = {}
                for s4 in range(4):
                    nb = int(sum(nblk_sh[t, s4] for t in range(t0, t1)))
                    if nb == 0:
                        continue
                    cl = calls[ci]
                    assert cl["sec"] == s4 and cl["cols"] == nb
                    ci += 1
                    gt = gpools[s4].tile([P, nb, ROWE], mybir.dt.bfloat16,
                                         tag=f"g{s4}")
                    ni = nb * P
                    nc.gpsimd.dma_gather(
                        gt[:], win(s4),
                        idx_t[:, l16:l16 + ni // 16],
                        ni, ni, ROWE,
                        single_packet=False, queue_num=qn % NQ)
                    qn += 1
                    off16 += ni // 16
                    l16 += ni // 16
                    c0 = cl["col0"]
                    chunks = []
                    h0 = 0
                    while h0 < nb:
                        hn = min((nb + 1) // 2, nb - h0)
                        m0b = mpools[s4].tile([P, hn, P], mybir.dt.bfloat16,
                                              tag=f"m{s4}")
                        nc.vector.tensor_tensor(
                            out=m0b[:],
                            in0=iota[:, None, :].broadcast_to([P, hn, P]),
                            in1=dstl_sb[:, c0 + h0:c0 + h0 + hn, None]
                            .broadcast_to([P, hn, P]),
                            op=AL.is_equal)
                        for r in range(h0, h0 + hn):
                            chunks.append((m0b, r - h0))
                        h0 += hn
                    g[s4] = (gt, c0, chunks)

                for t in range(t0, t1):
                    emit_tile(t, g)

            # ---- batched tail -------------------------------------------
            rden = cp.tile([P, NT], mybir.dt.float32)
            nc.vector.reciprocal(rden[:], aggbig[:, :, HIDDEN])
            ybig = bigp.tile([P, NT, HIDDEN], mybir.dt.float32, tag="ybig")
            nc.vector.tensor_tensor(
                out=ybig[:], in0=aggbig[:, :, 0:HIDDEN],
                in1=rden[:, :, None].broadcast_to([P, NT, HIDDEN]),
                op=AL.mult)
            nc.vector.tensor_tensor(
                out=ybig[:], in0=ybig[:],
                in1=bias_sb[:, None, :].broadcast_to([P, NT, HIDDEN]),
                op=AL.add)
            tmin = bigp.tile([P, NT, HIDDEN], mybir.dt.bfloat16, tag="tmin")
            nc.vector.tensor_scalar_min(out=tmin[:], in0=ybig[:], scalar1=0.0)
            nc.scalar.activation(out=tmin[:], in_=tmin[:], func=AF.Exp)
            nc.vector.tensor_scalar_max(out=ybig[:], in0=ybig[:], scalar1=0.0)
            nc.vector.tensor_scalar(out=tmin[:], in0=tmin[:], scalar1=1.0,
                                    scalar2=None, op0=AL.subtract)
            nc.vector.tensor_tensor(out=ybig[:], in0=ybig[:], in1=tmin[:],
                                    op=AL.add)
            for t in range(NT):
                yT_ps = pyp.tile([HIDDEN, P], mybir.dt.float32, space="PSUM",
                                 tag="yT")
                nc.tensor.transpose(out=yT_ps[:], in_=ybig[:, t, :],
                                    identity=ident[:])
                yT_sb = sp.tile([HIDDEN, P], mybir.dt.float32, tag="yT_sb")
                nc.vector.tensor_copy(out=yT_sb[:], in_=yT_ps[:])
                z_ps = pzp.tile([P, OUT_CH], mybir.dt.float32, space="PSUM",
                                tag="z")
                nc.tensor.matmul(out=z_ps[:], lhsT=yT_sb[:], rhs=linW_sb[:],
                                 start=True, stop=True)
                nc.vector.tensor_tensor(out=zst[:, t, :], in0=z_ps[:],
                                        in1=linb_sb[:], op=AL.add)
            nmx = cp.tile([P, NT], mybir.dt.float32)
            nc.vector.tensor_reduce(out=nmx[:], in_=zst[:],
                                    axis=mybir.AxisListType.X, op=AL.max)
            es = bigp.tile([P, NT, OUT_CH], mybir.dt.float32, tag="es")
            nc.vector.tensor_tensor(
                out=es[:], in0=zst[:],
                in1=nmx[:, :, None].broadcast_to([P, NT, OUT_CH]),
                op=AL.subtract)
            ex = bigp.tile([P, NT, OUT_CH], mybir.dt.bfloat16, tag="ex")
            nc.scalar.activation(out=ex[:], in_=es[:], func=AF.Exp)
            ssum = cp.tile([P, NT], mybir.dt.float32)
            nc.vector.tensor_reduce(out=ssum[:], in_=ex[:],
                                    axis=mybir.AxisListType.X, op=AL.add)
            lsum = cp.tile([P, NT], mybir.dt.float32)
            nc.scalar.activation(out=lsum[:], in_=ssum[:], func=AF.Ln)
            nc.vector.tensor_tensor(
                out=es[:], in0=es[:],
                in1=lsum[:, :, None].broadcast_to([P, NT, OUT_CH]),
                op=AL.subtract)
            nc.sync.dma_start(out=outz[:, :, :], in_=es[:])

    nc.finalize()
    return nc


EXEC_TIMES = []


def kernel(x, edge_index, W, att_src, att_dst, gat_bias, lin_W, lin_b):
    import os
    from concourse.bass_utils import run_bass_kernel_spmd
    trace = os.environ.get("GAT_TRACE") == "1"
    EXEC_TIMES.clear()

    x = np.asarray(x, _f32)
    edge_index = np.asarray(edge_index).astype(np.int64)
    W = np.asarray(W, _f32)
    att_src = np.asarray(att_src, _f32)
    att_dst = np.asarray(att_dst, _f32)
    gat_bias = np.asarray(gat_bias, _f32)
    lin_W = np.asarray(lin_W, _f32)
    lin_b = np.asarray(lin_b, _f32)

    # ---- phase A --------------------------------------------------------
    nc_a = _build_phase_a()
    xT = np.ascontiguousarray(x.T)
    att2 = np.stack([att_src, att_dst], axis=1)
    in_maps_a = []
    for c in range(CORES):
        sh = np.zeros((P, NT_A * P), _f32)
        sh[:, :NA] = xT[:, c * NA:(c + 1) * NA]
        in_maps_a.append({"xT": sh, "W": W, "att": att2})
    res_a = run_bass_kernel_spmd(nc_a, in_maps_a, core_ids=list(range(CORES)),
                                 trace=trace)
    EXEC_TIMES.append(("phase_a", res_a.exec_time_ns))

    NPAD = CORES * NT_A * P
    rowsP = np.zeros((NPAD, RU), _bf16)
    rowsM = np.zeros((NPAD, RU), _bf16)
    av = np.zeros((NPAD, 2), _f32)
    qv = np.zeros((NPAD, 2), _f32)
    for c in range(CORES):
        r = res_a.results[c]
        sl = slice(c * NA, (c + 1) * NA)
        rowsP[sl] = r["rowP"].transpose(1, 0, 2).reshape(-1, RU)[:NA]
        rowsM[sl] = r["rowM"].transpose(1, 0, 2).reshape(-1, RU)[:NA]
        av[sl] = r["avals"].transpose(1, 0, 2).reshape(-1, 2)[:NA]
        qv[sl] = r["qvals"].transpose(1, 0, 2).reshape(-1, 2)[:NA]
    rowsP, rowsM, av, qv = (rowsP[:N_NODES], rowsM[:N_NODES],
                            av[:N_NODES], qv[:N_NODES])

    tblP = np.zeros((N_NODES, ROWE), _bf16)
    tblP[:, 0:RU] = rowsP
    tblM = np.zeros((N_NODES, ROWE), _bf16)
    tblM[:, 0:RU] = rowsM

    # ---- host layout ----------------------------------------------------
    src, dst = edge_index[0], edge_index[1]
    sign = (av[src, 0] + av[dst, 1]) >= 0.0
    lay = _layout2(src, dst, sign)
    node_of = lay["node_of"]

    biasr = np.tile(gat_bias[None, :], (P, 1)).astype(_f32)
    linbr = np.tile(lin_b[None, :], (P, 1)).astype(_f32)

    in_maps_b = []
    for c in range(CORES):
        nm = node_of[c]
        nmc = np.where(nm >= 0, nm, 0)
        qstg = qv[nmc].transpose(1, 0, 2).astype(_f32)       # [P, NT, 2]
        ssign = ((av[nmc, 0] + av[nmc, 1]) >= 0.0)           # [NT, P]
        sR = np.where(ssign[:, :, None], rowsP[nmc], rowsM[nmc])
        qeffh = np.where(ssign, qv[nmc][:, :, 0], qv[nmc][:, :, 1])
        in_maps_b.append({
            "tblP": tblP, "tblM": tblM,
            "idxs": lay["idx"][c], "dstl": lay["dstl"][c],
            "qst": np.ascontiguousarray(qstg),
            "selfR": np.ascontiguousarray(sR.transpose(1, 0, 2)),
            "qeffh": np.ascontiguousarray(qeffh.T.astype(_f32)),
            "biasr": biasr, "linW": lin_W, "linbr": linbr,
        })

    nc_b = _build_phase_b(lay["calls"], lay["col_base"], lay["nblk_sh"],
                          lay["total_cols"])
    res_b = run_bass_kernel_spmd(nc_b, in_maps_b, core_ids=list(range(CORES)),
                                 trace=trace)
    EXEC_TIMES.append(("phase_b", res_b.exec_time_ns))

    out = np.zeros((N_NODES, OUT_CH), _f32)
    for c in range(CORES):
        oz = res_b.results[c]["outz"]                 # [P, NT, OUT_CH]
        nm = node_of[c]                               # [NT, P]
        valid = nm >= 0
        out[nm[valid]] = oz.transpose(1, 0, 2)[valid]
    return out
